# revision 1
# baseline (speedup 1.0000x reference)
"""Self-contained Trainium2 Bass kernel for nn_Encode_64579128262771.

6-pass shared-weight LSTM encoder (B=128, T=256, H=512):
  pass 0:   lstm(x,  W1,U1,b1, 0,0)
  pass 1-4: lstm(seq,W1,U1,b1, h,c)   (same weights, chained state)
  pass 5:   lstm(seq,W2,U2,b2, h,c) -> returns last h

Sharding: 8-way data-parallel over batch (16 rows/core), chosen because the
recurrence is sequential in time AND across passes (each pass starts from the
previous pass's final state), and this toolchain supports no usable low-latency
cross-core exchange (collectives floor ~5us/step; remote_dma not supported by
this walrus build). Matmuls run as float32r (full PE rate, ~1.5e-4 rel err).
Per pass: a dense pre-GEMM computes zx = seq_in @ W + b for all timesteps
(bias folded in as an extra K row against a constant ones row; tanh(g) folded
into the sigmoid pass by pre-scaling g columns by 2), then the sequential
phase computes z = zx + h@U, gates, cell update, and h^T via PE transpose.
"""

import sys

sys.path.insert(0, "/opt/trn_rl_repo")

import numpy as np

_KL_SRC = None  # populated below


def _get_lib():
    global _KL_SRC
    import importlib

    if _KL_SRC is None:
        _KL_SRC = importlib.import_module("kernel_embedded_lib")
    return _KL_SRC


# The kernel library is embedded in this file's directory as a sibling would
# not survive the fresh-directory grading harness; instead we inline it via
# exec of the source string stored below at import time.

import types as _types

_LIB_SOURCE = r'''
"""Bass/Trainium2 kernel for the 6-pass shared-weight LSTM encoder.

Strategy (decided after microbenchmarking this container's toolchain):
  - 8-way data parallel over batch (B=128 -> 16 rows/core). The recurrence is
    sequential in time and across passes (each pass consumes the previous
    pass's final (h, c) as its initial state), so there is no cross-core
    parallelism available for the recurrent part without per-step
    communication -- and both ncfw collectives (~5us floor) and remote_dma
    (unsupported by this walrus build: "ISA wrong length") are unusable here.
  - float32r matmuls: measured ~300ns for [128x128]x[128x512] (same as bf16)
    with 15x better accuracy (rel err ~1.6e-4), fp32 storage -> no dtype
    conversions anywhere.
  - Per pass: phase A precomputes zx = in_seq @ W + b as a dense GEMM (the
    x@W half of the cell is not sequential); phase B runs the 256 recurrent
    steps computing z = zx + h@U, gates, cell update, and h^T via PE
    transpose (h^T feeds the next step's matmul as the stationary operand).
  - The bias add and the tanh(g) gate are folded away host-side: b becomes an
    extra K-row of W against a constant ones-column of the input, and the g
    columns of W/U/b are pre-scaled by 2 so tanh(zg) = 2*sigmoid(2*zg) - 1
    comes out of the same sigmoid pass as i, f, o (one DVE affine op).
"""

import sys

sys.path.insert(0, "/opt/trn_rl_repo")

import numpy as np

import concourse.bass as bass
import concourse.mybir as mybir
from concourse.tile import TileContext
from bass_rust import ScopedClock

F32 = mybir.dt.float32
F32R = mybir.dt.float32r
AF = mybir.ActivationFunctionType
ALU = mybir.AluOpType

# ---------------------------------------------------------------------------
# Toolchain workarounds
# ---------------------------------------------------------------------------

_SPLIT_CTR = [0]


def split_multiwaits(nc):
    """This container's walrus encodes at most ONE semaphore wait per
    instruction; hoist extra waits onto single-wait NoOps just before the
    instruction (same engine, so semantics are identical)."""
    n_fixed = 0
    for f in nc.m.functions:
        for bb in f.blocks:
            insts = bb.instructions
            if not any(
                i.sync_info is not None
                and i.sync_info.on_wait
                and len(i.sync_info.on_wait) > 1
                for i in insts
            ):
                continue
            newl = []
            for inst in insts:
                si = inst.sync_info
                if si is not None and si.on_wait and len(si.on_wait) > 1:
                    waits = list(si.on_wait)
                    for w in waits[:-1]:
                        _SPLIT_CTR[0] += 1
                        nop = mybir.InstNoOp(
                            name=f"I-mwsplit-{_SPLIT_CTR[0]}", ins=[], outs=[]
                        )
                        nop.engine = inst.engine
                        nop.sync_info = mybir.SyncInfo(on_wait=[w], on_update=[])
                        newl.append(nop)
                    inst.sync_info = mybir.SyncInfo(
                        on_wait=[waits[-1]], on_update=list(si.on_update)
                    )
                    n_fixed += 1
                newl.append(inst)
            bb.instructions = newl
    return n_fixed


def _patched_drain_and_barrier(self, tick_clock, wait_clock):
    """Same as TileContext._drain_and_barrier but emits the final global-clock
    waits as individual single-wait SP instructions (1-wait ISA limit)."""
    nc = self.nc
    probe = mybir.InstNoOp(name="I-probe-drain", ins=[], outs=[])
    probe.engine = mybir.EngineType.SP
    wait_clock.add_sem_waits(probe, ScopedClock({None: tick_clock.global_clock}))
    si = probe.sync_info
    waits = list(si.on_wait) if si is not None else []
    id2handle = {h.num: h for h in self.sems.allocated().values()}
    for w in waits:
        h = id2handle.get(w.id)
        assert h is not None, f"no handle for sem id {w.id} ({w.ant_name})"
        nc.sync.wait_ge(h, w.wait_value)
    nc.sync.drain()
    nc.all_engine_barrier()
    popped = nc._tile_sem_poison_stack.pop()
    assert popped is self._sem_poison
    nc.clear_and_free_semaphores(list(self.sems.allocated().values()))
    nc.all_engine_barrier()


TileContext._drain_and_barrier = _patched_drain_and_barrier


def install_ntff_hook():
    """Register the NTFF profiling hook that trn_boot skipped because the
    image's antenv package lacks axon_hooks (needed for trace=True)."""
    import types

    if "antenv.axon_hooks" in sys.modules:
        return
    mod = types.ModuleType("antenv.axon_hooks")
    mod._hook = None

    def _set(h):
        mod._hook = h

    def _get():
        return mod._hook

    mod.set_axon_ntff_profile_hook = _set
    mod.get_axon_ntff_profile_hook = _get
    import antenv

    sys.modules["antenv.axon_hooks"] = mod
    antenv.axon_hooks = mod
    try:
        from trn_agent_boot.trn_boot import _ntff_profile_via_ctypes

        _set(_ntff_profile_via_ctypes("/opt/axon/libaxon_pjrt.so"))
    except Exception:
        pass
    import concourse.bass_utils as bu

    bu.upload_artifacts = lambda tmpdir: tmpdir


# ---------------------------------------------------------------------------
# Host-side weight preparation
# ---------------------------------------------------------------------------

H = 512
G4 = 4 * H  # 2048
KX = H + 128  # padded input K (ones row at index H, zeros after)
NSTRIP = 4  # H is processed in 4 slices of 128


def strip_permutation():
    """Column permutation of the 4H gate dim: strip j holds
    [i_j | f_j | g_j | o_j], each 128 wide, for H-dims [128j, 128j+128)."""
    perm = []
    for j in range(NSTRIP):
        for gate in range(4):
            base = gate * H + j * 128
            perm.extend(range(base, base + 128))
    return np.array(perm, dtype=np.int64)


def prep_weights(W, U, b):
    """Returns (Wb [KX, G4], Ur [4, 128, G4]) with strip-permuted columns,
    g columns pre-scaled by 2, and b folded as row H of Wb."""
    perm = strip_permutation()
    scale = np.ones((G4,), np.float32)
    scale[2 * H : 3 * H] = 2.0  # g gate block in ORIGINAL column order
    Wp = (W * scale[None, :])[:, perm]
    Up = (U * scale[None, :])[:, perm]
    bp = (b * scale)[perm]
    Wb = np.zeros((KX, G4), np.float32)
    Wb[:H] = Wp
    Wb[H] = bp
    Ur = np.ascontiguousarray(Up.reshape(4, 128, G4))
    return Wb, Ur


def prep_xbT(x_slice, T):
    """x_slice [16, T, 512] -> xbT [5, 128, T*16] (K-major, col = t*16 + b),
    with the ones row at K index 512 (block 4, row 0)."""
    Bc = x_slice.shape[0]
    xt = np.transpose(x_slice, (2, 1, 0))  # [512, T, 16]
    out = np.zeros((5, 128, T, Bc), np.float32)
    out[:4] = xt.reshape(4, 128, T, Bc)
    out[4, 0, :, :] = 1.0
    return out


# ---------------------------------------------------------------------------
# Kernel builder
# ---------------------------------------------------------------------------


def r(ap):
    return ap.bitcast(F32R)


def build_kernel(T=256, n_mid=4, col_tiled=True, unroll=8, gemm_unroll=2, static_loops=False):
    """Build the full 6-pass kernel program. Returns nc.

    Inputs (per core): xbT [5,128,T*16], wb1/wb2 [KX? stored as [5,128,2048]],
    ur1/ur2 [4,128,2048], i16 [16,16].
    Output: ht_out [128, 64]  (h^T chunks side by side; host reassembles).
    """
    Bc = 16
    NT = T * Bc  # columns of the seq buffers
    MT = NT // 128  # number of 128-row M-tiles in the pre-GEMM

    nc = bass.Bass()

    xbT = nc.declare_dram_parameter("xbT", [5, 128, T, 16], F32R, isOutput=False)
    wb1 = nc.declare_dram_parameter("wb1", [5, 128, G4], F32R, isOutput=False)
    wb2 = nc.declare_dram_parameter("wb2", [5, 128, G4], F32R, isOutput=False)
    ur1 = nc.declare_dram_parameter("ur1", [4, 128, G4], F32R, isOutput=False)
    ur2 = nc.declare_dram_parameter("ur2", [4, 128, G4], F32R, isOutput=False)
    i16d = nc.declare_dram_parameter("i16", [16, 16], F32R, isOutput=False)
    ht_out = nc.declare_dram_parameter("ht_out", [128, 64], F32R, isOutput=True)

    with TileContext(nc) as tc:
        dram = tc.alloc_tile_pool(name="dram", bufs=1, space="DRAM")
        seqT = dram.tile([5, 128, T, 16], F32R)
        zx = dram.tile([T * 16, G4], F32R)

        wpool = tc.alloc_tile_pool(name="w", bufs=1)
        wbS1 = wpool.tile([128, 5, G4], F32R)
        urS1 = wpool.tile([128, 4, G4], F32R)
        i16 = wpool.tile([16, 16], F32R)
        i16f = wpool.tile([16, 16], F32)

        state = tc.alloc_tile_pool(name="state", bufs=1)
        hT = state.tile([128, 64], F32R)  # h^T chunk j at cols [16j, 16j+16)
        cS = state.tile([128, 128] if col_tiled else [16, H], F32)  # cell state

        # pools for the loop bodies
        alhs = tc.alloc_tile_pool(name="alhs", bufs=2)
        apsum = tc.alloc_tile_pool(name="apsum", bufs=2, space="PSUM")
        zxs = tc.alloc_tile_pool(name="zxs", bufs=2)
        zpsum = tc.alloc_tile_pool(name="zpsum", bufs=2 if col_tiled else 1, space="PSUM")
        gates = tc.alloc_tile_pool(name="gates", bufs=2)
        gsc = tc.alloc_tile_pool(name="gsc", bufs=2)
        tpsum = tc.alloc_tile_pool(name="tpsum", bufs=2, space="PSUM")

        # one-time loads / init
        for kc in range(5):
            nc.sync.dma_start(out=wbS1[:, kc, :], in_=wb1[kc])
        for kc in range(4):
            nc.sync.dma_start(out=urS1[:, kc, :], in_=ur1[kc])
        nc.sync.dma_start(out=i16[:], in_=i16d[:])
        nc.sync.dma_start(out=i16f[:], in_=i16d[:].bitcast(F32))
        nc.vector.memset(hT[:].bitcast(F32), 0.0)
        nc.vector.memset(cS[:], 0.0)
        # seq buffers: ones row at block 4 row 0 (once; kernel never writes it)
        TBLK = min(T, 32)
        onesblk = wpool.tile([128, TBLK * 16], F32R)
        nc.vector.memset(onesblk[:].bitcast(F32), 0.0)
        nc.vector.memset(onesblk[0:1, :].bitcast(F32), 1.0)
        for t0 in range(0, T, TBLK):
            nc.sync.dma_start(
                out=seqT[4, :, t0 : t0 + TBLK],
                in_=onesblk[:].rearrange("p (a b) -> p a b", a=TBLK),
            )

        def phase_a(inT, wbS, tag, rd_eng, wr_eng, force_static=False):
            """zx[:] = in_seq @ Wb  (dense GEMM over all T*16 rows)."""

            def body(mt):
                lhsT = alhs.tile([128, 5, 128], F32R, tag="alhs")
                rd_eng.dma_start(
                    out=lhsT[:].rearrange("p k (a b) -> p k a b", a=8),
                    in_=inT[0:5, :, bass.ds(mt * 8, 8)].rearrange("k p a b -> p k a b"),
                )
                zst = alhs.tile([128, G4], F32R, tag="zst")
                for bk in range(4):
                    ps = apsum.tile([128, 512], F32, tag="apsum")
                    for kc in range(5):
                        nc.tensor.matmul(
                            ps[:],
                            r(lhsT[:, kc, :]),
                            r(wbS[:, kc, bass.ts(bk, 512)]),
                            start=(kc == 0),
                            stop=(kc == 4),
                        )
                    nc.vector.tensor_copy(zst[:, bass.ts(bk, 512)], ps[:])
                wr_eng.dma_start(out=zx[bass.ds(mt * 128, 128), :], in_=zst[:])

            if static_loops or force_static:
                for _mt in range(MT):
                    body(_mt)
            else:
                tc.For_i_unrolled(0, MT, 1, body, max_unroll=gemm_unroll)

        def phase_b(wr_seq, urS, tag, rd_eng, wr_eng):
            """T recurrent steps in blocks of SB=8. One zx read and one seq
            write per block; h^T flows through a staging strip so consecutive
            steps pipeline without a WAR on a single state tile."""
            SB = 4

            def body(t0):
                zx8 = zxs.tile([16, SB, G4], F32R, tag="zxs")
                rd_eng.dma_start(
                    out=zx8[:],
                    in_=zx[bass.ds(t0 * 16 * SB, 16 * SB), :].rearrange(
                        "(u b) n -> b u n", b=16
                    ),
                )
                stage = gates.tile([128, SB * 64], F32R, tag="stage")
                for u in range(SB):

                    def hsrc(k):
                        if u == 0:
                            return hT[:, bass.ts(k, 16)]
                        return stage[:, bass.ds(16 * (k * SB + u - 1), 16)]

                    P = zpsum.tile([16, G4], F32, tag="zp")
                    for j in range(4):
                        nc.tensor.matmul(
                            P[:, bass.ts(j, 512)],
                            r(i16[:]),
                            r(zx8[:, u, bass.ts(j, 512)]),
                            start=True,
                            stop=False,
                        )
                    for k in range(4):
                        for j in range(4):
                            nc.tensor.matmul(
                                P[:, bass.ts(j, 512)],
                                r(hsrc(k)),
                                r(urS[:, k, bass.ts(j, 512)]),
                                start=False,
                                stop=(k == 3),
                            )
                    sig = gates.tile([16, G4], F32, tag="sig")
                    nc.scalar.activation(sig[:], P[:], AF.Sigmoid)

                    def sg(j, gate):
                        return sig[:, bass.ds(512 * j + 128 * gate, 128)]

                    for j in range(4):
                        sl = bass.ts(j, 128)
                        gp = gsc.tile([16, 128], F32, tag="gp")
                        m1 = gsc.tile([16, 128], F32, tag="m1")
                        m2 = gsc.tile([16, 128], F32, tag="m2")
                        th = gsc.tile([16, 128], F32, tag="th")
                        nc.vector.tensor_scalar(gp[:], sg(j, 2), 2.0, -1.0, ALU.mult, ALU.add)
                        nc.vector.tensor_tensor(m1[:], sg(j, 1), cS[0:16, sl], ALU.mult)
                        nc.vector.tensor_tensor(m2[:], sg(j, 0), gp[:], ALU.mult)
                        nc.vector.tensor_tensor(cS[0:16, sl], m1[:], m2[:], ALU.add)
                        nc.scalar.activation(th[:], cS[0:16, sl], AF.Tanh)
                        hN = gsc.tile([16, 128], F32, tag="hN")
                        nc.vector.tensor_tensor(hN[:], sg(j, 3), th[:], ALU.mult)
                        PT = tpsum.tile([128, 16], F32, tag="pt")
                        nc.tensor.transpose(PT[:], hN[:], i16f[:])
                        nc.vector.tensor_copy(
                            stage[:, bass.ds(16 * (j * SB + u), 16)], PT[:]
                        )
                nc.vector.tensor_copy(
                    hT[:].rearrange("p (c b) -> p c b", c=4),
                    stage[:].rearrange("p (c u b) -> p c u b", c=4, u=SB)[:, :, SB - 1],
                )
                if wr_seq:
                    wr_eng.dma_start(
                        out=seqT[0:4, :, bass.ds(t0 * SB, SB)].rearrange(
                            "c p u b -> p c u b"
                        ),
                        in_=stage[:].rearrange("p (c u b) -> p c u b", c=4, u=SB),
                    )

            if static_loops:
                for _t in range(T // SB):
                    body(_t)
            else:
                tc.For_i_unrolled(0, T // SB, 1, body, max_unroll=max(1, unroll // SB))

        # pass 0 pre-GEMM from x
        phase_a(xbT, wbS1, "a0", nc.sync, nc.scalar, force_static=True)
        if n_mid < 0:  # debug: single pass only
            zxdbg = nc.declare_dram_parameter("zx_dbg", [16, G4], F32R, isOutput=True)
            dbt = alhs.tile([16, G4], F32R, tag="dbg")
            nc.sync.dma_start(out=dbt[:], in_=zx[bass.ds(0, 16), :])
            nc.sync.dma_start(out=zxdbg[:], in_=dbt[:])
            phase_b(False, urS1, "bdbg", nc.sync, nc.scalar)
            nc.sync.dma_start(out=ht_out[:], in_=hT[:])
            for pool in (tpsum, gsc, gates, zpsum, zxs, apsum, alhs, state, wpool, dram):
                pool.release()
            _single = True
        else:
            _single = False
        # passes 0..n_mid-1 fused with the next pass's pre-GEMM in one loop
        if _single:
            pass
        elif n_mid > 0:
            with tc.For_i(0, n_mid, 1) as _p:
                phase_b(True, urS1, "bmid", nc.sync, nc.scalar)
                phase_a(seqT, wbS1, "amid", nc.sync, nc.scalar)
        # pass n_mid (last shared-weight pass) + final pre-GEMM (W2) + final pass
        if not _single:
            phase_b(True, urS1, "blast1", nc.sync, nc.scalar)
            # overwrite W weights with the final pass's W2 (phase order is
            # serialized by Tile's WAR tracking on wbS1)
            for kc in range(5):
                nc.sync.dma_start(out=wbS1[:, kc, :], in_=wb2[kc])
            phase_a(seqT, wbS1, "alast", nc.sync, nc.scalar, force_static=True)
            # likewise swap in U2 before the final recurrence
            for kc in range(4):
                nc.sync.dma_start(out=urS1[:, kc, :], in_=ur2[kc])
            phase_b(False, urS1, "bfin", nc.sync, nc.scalar)
            nc.sync.dma_start(out=ht_out[:], in_=hT[:])
            for pool in (tpsum, gsc, gates, zpsum, zxs, apsum, alhs, state, wpool, dram):
                pool.release()

    nfix = split_multiwaits(nc)
    return nc, nfix


def make_inputs_for_core(x_core, W1, U1, b1, W2, U2, b2, T=256):
    Wb1, Ur1 = prep_weights(W1, U1, b1)
    Wb2, Ur2 = prep_weights(W2, U2, b2)
    return {
        "xbT": prep_xbT(x_core, T),
        "wb1": np.ascontiguousarray(Wb1.reshape(5, 128, G4)),
        "wb2": np.ascontiguousarray(Wb2.reshape(5, 128, G4)),
        "ur1": Ur1,
        "ur2": Ur2,
        "i16": np.eye(16, dtype=np.float32),
    }


def assemble_h(ht_res):
    """ht_res [128, 64] -> h [16, 512]: h[b, 128j + r] = ht[r, 16j + b]."""
    ht = ht_res.reshape(128, 4, 16)
    return np.transpose(ht, (2, 1, 0)).reshape(16, 512)

'''

_mod = _types.ModuleType("kernel_embedded_lib")
exec(compile(_LIB_SOURCE, "kernel_embedded_lib", "exec"), _mod.__dict__)
sys.modules["kernel_embedded_lib"] = _mod

_BUILT = {}


def _build(col_tiled=False):
    key = col_tiled
    if key not in _BUILT:
        nc, _ = _mod.build_kernel(
            T=256, n_mid=4, col_tiled=col_tiled, unroll=8, gemm_unroll=2
        )
        _BUILT[key] = nc
    return _BUILT[key]


def kernel(x, W1, U1, b1, W2, U2, b2):
    """Full inputs in, full output out. x [128,256,512] fp32 -> h [128,512]."""
    from concourse.bass_utils import run_bass_kernel_spmd

    x = np.asarray(x, dtype=np.float32)
    nc = _build()
    in_maps = []
    for c in range(8):
        xc = x[c * 16 : (c + 1) * 16]
        in_maps.append(
            _mod.make_inputs_for_core(
                xc,
                np.asarray(W1, np.float32),
                np.asarray(U1, np.float32),
                np.asarray(b1, np.float32),
                np.asarray(W2, np.float32),
                np.asarray(U2, np.float32),
                np.asarray(b2, np.float32),
                T=256,
            )
        )
    res = run_bass_kernel_spmd(nc, in_maps, list(range(8)))
    parts = [_mod.assemble_h(res.results[i]["ht_out"]) for i in range(8)]
    return np.concatenate(parts, axis=0)



# revision 2
# speedup vs baseline: 1.7561x; 1.7561x over previous
"""Self-contained Trainium2 Bass kernel for nn_Encode_64579128262771 (v2).

8-way data-parallel over batch (16 rows/core); per core a 6-pass LSTM with
interleaved dense pre-GEMM, SBUF zx ring, half-split tails. See kernel_lib.
"""

import os
import sys

sys.path.insert(0, "/opt/trn_rl_repo")

import numpy as np

_HERE = os.path.dirname(os.path.abspath(__file__))

_LIB_PATH = os.path.join(_HERE, "kernel_lib.py")
_LIB_SOURCE = None
if os.path.exists(_LIB_PATH):
    _LIB_SOURCE = open(_LIB_PATH).read()
else:
    _LIB_SOURCE = None  # replaced by embedded copy in the final version

import types as _types

_mod = _types.ModuleType("kernel_embedded_lib")
exec(compile(_LIB_SOURCE, "kernel_embedded_lib", "exec"), _mod.__dict__)
sys.modules["kernel_embedded_lib"] = _mod

_BUILT = {}


def _build():
    if "nc" not in _BUILT:
        _BUILT["nc"] = _mod.build_kernel()
    return _BUILT["nc"]


def kernel(x, W1, U1, b1, W2, U2, b2):
    """Full inputs in, full output out. x [128,256,512] fp32 -> h [128,512]."""
    from concourse.bass_utils import run_bass_kernel_spmd

    x = np.asarray(x, dtype=np.float32)
    nc = _build()
    in_maps = []
    for c in range(8):
        in_maps.append(
            _mod.make_inputs_for_core(
                x[c * 16 : (c + 1) * 16],
                np.asarray(W1, np.float32),
                np.asarray(U1, np.float32),
                np.asarray(b1, np.float32),
                np.asarray(W2, np.float32),
                np.asarray(U2, np.float32),
                np.asarray(b2, np.float32),
            )
        )
    res = run_bass_kernel_spmd(nc, in_maps, list(range(8)))
    parts = [_mod.assemble_h(res.results[i]["ht_out"]) for i in range(8)]
    return np.concatenate(parts, axis=0)


# revision 3
# speedup vs baseline: 1.8012x; 1.0257x over previous
"""Self-contained Trainium2 Bass kernel for nn_Encode_64579128262771 (v2).

8-way data-parallel over batch (16 rows/core); per core a 6-pass LSTM with
interleaved dense pre-GEMM (SBUF zx ring), gate-major half-bank recurrence
matmuls, half-split tails, and batched seq DMAs. Embedded library below.
"""

import sys

sys.path.insert(0, "/opt/trn_rl_repo")

import types as _types

import numpy as np

_LIB_SOURCE = r'''
"""Bass/Trainium2 kernel v2 for the 6-pass shared-weight LSTM encoder.

Structure per core (16 batch rows, f32r matmuls):
  - 6 sequential passes (state chains), each: 256 recurrence steps.
  - zx = seq_in @ W + b computed as a dense GEMM in chunks of 8 steps,
    interleaved into the recurrence's PE gaps, output kept in an SBUF
    ring (2 slots of [128, 2048]) -- no DRAM roundtrip for zx.
  - Per step: 8 half-bank MM groups (selector-inject + 4 k-matmuls,
    N=256, gate-major cols [i|f|g|o]); sigma/tanh fire per group as its
    PSUM bank completes (overlapping the stream); half-split back-tail
    (m2/m1/c/tanh/h per 256-dim half) so transposes fire early;
    h-transposes as regular tiny matmuls (lhsT=h chunk, rhs=I16, N=16).
  - seq output (h^T) DMA'd to DRAM seqT buffers for the next pass's
    dense GEMM input.
"""

import sys

sys.path.insert(0, "/opt/trn_rl_repo")

import numpy as np

import concourse.bass as bass
import concourse.mybir as mybir
from concourse.tile import TileContext
from bass_rust import ScopedClock

F32 = mybir.dt.float32
F32R = mybir.dt.float32r
AF = mybir.ActivationFunctionType
ALU = mybir.AluOpType

H = 512
G4 = 4 * H
T = 256

# ---------------------------------------------------------------------------
# Toolchain workarounds (same as baseline)
# ---------------------------------------------------------------------------

_SPLIT_CTR = [0]


def split_multiwaits(nc):
    n_fixed = 0
    for f in nc.m.functions:
        for bb in f.blocks:
            insts = bb.instructions
            if not any(
                i.sync_info is not None
                and i.sync_info.on_wait
                and len(i.sync_info.on_wait) > 1
                for i in insts
            ):
                continue
            newl = []
            for inst in insts:
                si = inst.sync_info
                if si is not None and si.on_wait and len(si.on_wait) > 1:
                    waits = list(si.on_wait)
                    for w in waits[:-1]:
                        _SPLIT_CTR[0] += 1
                        nop = mybir.InstNoOp(
                            name=f"I-mwsplit-{_SPLIT_CTR[0]}", ins=[], outs=[]
                        )
                        nop.engine = inst.engine
                        nop.sync_info = mybir.SyncInfo(on_wait=[w], on_update=[])
                        newl.append(nop)
                    inst.sync_info = mybir.SyncInfo(
                        on_wait=[waits[-1]], on_update=list(si.on_update)
                    )
                    n_fixed += 1
                newl.append(inst)
            bb.instructions = newl
    return n_fixed


def _patched_drain_and_barrier(self, tick_clock, wait_clock):
    nc = self.nc
    probe = mybir.InstNoOp(name="I-probe-drain", ins=[], outs=[])
    probe.engine = mybir.EngineType.SP
    wait_clock.add_sem_waits(probe, ScopedClock({None: tick_clock.global_clock}))
    si = probe.sync_info
    waits = list(si.on_wait) if si is not None else []
    id2handle = {h.num: h for h in self.sems.allocated().values()}
    for w in waits:
        h = id2handle.get(w.id)
        assert h is not None, f"no handle for sem id {w.id} ({w.ant_name})"
        nc.sync.wait_ge(h, w.wait_value)
    nc.sync.drain()
    nc.all_engine_barrier()
    popped = nc._tile_sem_poison_stack.pop()
    assert popped is self._sem_poison
    nc.clear_and_free_semaphores(list(self.sems.allocated().values()))
    nc.all_engine_barrier()


TileContext._drain_and_barrier = _patched_drain_and_barrier


def install_ntff_hook():
    import types

    if "antenv.axon_hooks" in sys.modules:
        return
    mod = types.ModuleType("antenv.axon_hooks")
    mod._hook = None

    def _set(h):
        mod._hook = h

    def _get():
        return mod._hook

    mod.set_axon_ntff_profile_hook = _set
    mod.get_axon_ntff_profile_hook = _get
    import antenv

    sys.modules["antenv.axon_hooks"] = mod
    antenv.axon_hooks = mod
    try:
        from trn_agent_boot.trn_boot import _ntff_profile_via_ctypes

        _set(_ntff_profile_via_ctypes("/opt/axon/libaxon_pjrt.so"))
    except Exception:
        pass
    import concourse.bass_utils as bu

    bu.upload_artifacts = lambda tmpdir: tmpdir


# ---------------------------------------------------------------------------
# Host-side prep
# ---------------------------------------------------------------------------


def prep_weights(W, U, b):
    """Wb [5,128,G4] (bias folded as chunk4 row0), Ur [128,4,G4].
    Gate-major columns (keras order i|f|g|o already)."""
    Wb = np.zeros((5, 128, G4), np.float32)
    Wb[:4] = W.reshape(4, 128, G4)
    Wb[4, 0] = b
    Ur = np.ascontiguousarray(U.reshape(4, 128, G4).transpose(1, 0, 2))
    return Wb, Ur


def prep_xbT(x_slice):
    """x_slice [16, T, 512] -> [4, 128, T, 16] (k-chunk, dim, t, b)."""
    xt = np.transpose(x_slice, (2, 1, 0))  # [512, T, 16]
    return np.ascontiguousarray(xt.reshape(4, 128, T, 16))


def make_esel():
    e = np.zeros((128, 8, 16), np.float32)
    for u in range(8):
        e[16 * u : 16 * (u + 1), u] = np.eye(16, dtype=np.float32)
    return e


def assemble_h(ht_res):
    """ht [128, 64] -> h [16, 512]: h[b, 128k + d] = ht[d, 16k + b]."""
    ht = ht_res.reshape(128, 4, 16)
    return np.transpose(ht, (2, 1, 0)).reshape(16, 512)


# ---------------------------------------------------------------------------
# Kernel builder
# ---------------------------------------------------------------------------


def r(ap):
    return ap.bitcast(F32R)


def build_kernel():
    nc = bass.Bass()

    xbT = nc.declare_dram_parameter("xbT", [4, 128, T, 16], F32R, isOutput=False)
    wb1 = nc.declare_dram_parameter("wb1", [5, 128, G4], F32R, isOutput=False)
    wb2 = nc.declare_dram_parameter("wb2", [5, 128, G4], F32R, isOutput=False)
    ur1 = nc.declare_dram_parameter("ur1", [128, 4, G4], F32R, isOutput=False)
    ur2 = nc.declare_dram_parameter("ur2", [128, 4, G4], F32R, isOutput=False)
    eseld = nc.declare_dram_parameter("esel", [128, 8, 16], F32R, isOutput=False)
    i16d = nc.declare_dram_parameter("i16", [16, 16], F32R, isOutput=False)
    ht_out = nc.declare_dram_parameter("ht_out", [128, 64], F32R, isOutput=True)

    GATE_I, GATE_F, GATE_G, GATE_O = 0, 1, 2, 3
    GATES = (GATE_I, GATE_G, GATE_F, GATE_O)

    with TileContext(nc) as tc:
        dram = tc.alloc_tile_pool(name="dram", bufs=1, space="DRAM")
        seqT = [dram.tile([4, 128, T, 16], F32R, name=f"seqT{i}") for i in range(2)]

        wpool = tc.alloc_tile_pool(name="w", bufs=1)
        w1sb = wpool.tile([128, 5, G4], F32R)
        u1sb = wpool.tile([128, 4, G4], F32R)
        w2sb = wpool.tile([128, 5, G4], F32R)
        u2sb = wpool.tile([128, 4, G4], F32R)
        ringA = wpool.tile([128, G4], F32R)
        ringB = wpool.tile([128, G4], F32R)
        esel = wpool.tile([128, 8, 16], F32R)
        i16 = wpool.tile([16, 16], F32R)
        ones128 = wpool.tile([128, 128], F32R)
        cS = wpool.tile([16, H], F32)
        stageA = wpool.tile([128, 64], F32R)
        stageB = wpool.tile([128, 64], F32R)

        sigp = tc.alloc_tile_pool(name="sig", bufs=2)
        mp = tc.alloc_tile_pool(name="m", bufs=2)
        hp = tc.alloc_tile_pool(name="h", bufs=2)
        lhp = tc.alloc_tile_pool(name="lh", bufs=1)
        sqb = tc.alloc_tile_pool(name="sqb", bufs=2)
        ppool = tc.alloc_tile_pool(name="P", bufs=4, space="PSUM")
        tpool = tc.alloc_tile_pool(name="PT", bufs=1, space="PSUM")
        dpool = tc.alloc_tile_pool(name="D", bufs=2, space="PSUM")

        # one-time loads / init
        for kc in range(5):
            nc.sync.dma_start(out=w1sb[:, kc, :], in_=wb1[kc])
            nc.sync.dma_start(out=w2sb[:, kc, :], in_=wb2[kc])
        nc.sync.dma_start(out=u1sb[:], in_=ur1[:])
        nc.sync.dma_start(out=u2sb[:], in_=ur2[:])
        nc.sync.dma_start(out=esel[:], in_=eseld[:])
        nc.sync.dma_start(out=i16[:], in_=i16d[:])
        nc.vector.memset(ones128[:].bitcast(F32), 0.0)
        nc.vector.memset(ones128[0:1, :].bitcast(F32), 1.0)
        nc.vector.memset(cS[:], 0.0)
        nc.vector.memset(stageA[:].bitcast(F32), 0.0)
        nc.vector.memset(stageB[:].bitcast(F32), 0.0)

        rings = [ringA, ringB]
        stages = [stageA, stageB]

        def dense_emit(wsb, lhsD, arow, slot, quarters):
            """MM quarter-groups for dense chunk rows lhsD[:, :, arow:arow+8]
            into ring[slot]."""
            ring = rings[slot]
            for q in quarters:
                psD = dpool.tile([128, 512], F32, name="psD", tag="psD")
                for kc in range(5):
                    lh = (
                        ones128[:]
                        if kc == 4
                        else lhsD[:, kc, arow : arow + 8].rearrange("p a b -> p (a b)")
                    )
                    nc.tensor.matmul(
                        psD[:], lh, r(wsb[:, kc, 512 * q : 512 * q + 512]),
                        start=(kc == 0), stop=(kc == 4),
                    )
                nc.vector.tensor_copy(ring[:, 512 * q : 512 * q + 512], psD[:])

        par = [0]  # global step parity for stageA/B alternation

        def step_emit(usb, s16, seqbuf, dense_job):
            """One recurrence step. s16: step index within 16-step body
            (static); seqbuf: [128,8,64] staging for seq writes or None;
            dense_job: (src, wsb, m_ap, slot, quarters, do_dma) or None."""
            u = s16 % 8
            slot = s16 // 8
            ring = rings[slot]
            stage_in = stages[par[0]]
            stage_out = stages[1 - par[0]]
            par[0] = 1 - par[0]

            sig = sigp.tile([16, 4, 512], F32, name="sig", tag="sig")

            def sg(gate):
                return sig[:, gate, :]

            for half in (0, 1):
                for g in GATES:
                    c0 = 512 * g + 256 * half
                    P = ppool.tile([16, 256], F32, name="P", tag="P")
                    nc.tensor.matmul(
                        P[:], r(esel[:, u, :]), r(ring[:, c0 : c0 + 256]),
                        start=True, stop=False,
                    )
                    for k in range(4):
                        nc.tensor.matmul(
                            P[:], r(stage_in[:, 16 * k : 16 * k + 16]),
                            r(usb[:, k, c0 : c0 + 256]),
                            start=False, stop=(k == 3),
                        )
                    fn = AF.Tanh if g == GATE_G else AF.Sigmoid
                    nc.scalar.activation(
                        sg(g)[:, 256 * half : 256 * half + 256], P[:], fn
                    )

            # dense quarter-group rides between the stream and the tail
            if dense_job is not None:
                dense_emit(*dense_job)

            hN = hp.tile([16, H], F32R, name="hN", tag="h")
            PT = tpool.tile([128, 64], F32, name="PT", tag="PT")
            for half in (0, 1):
                hc = slice(256 * half, 256 * half + 256)
                m2 = mp.tile([16, 256], F32, name="m2", tag=f"m2{half}")
                m1 = mp.tile([16, 256], F32, name="m1", tag=f"m1{half}")
                th = mp.tile([16, 256], F32, name="th", tag=f"th{half}")
                nc.vector.tensor_tensor(m2[:], sg(GATE_I)[:, hc], sg(GATE_G)[:, hc], ALU.mult)
                nc.vector.tensor_tensor(m1[:], sg(GATE_F)[:, hc], cS[:, hc], ALU.mult)
                nc.vector.tensor_tensor(cS[:, hc], m1[:], m2[:], ALU.add)
                nc.scalar.activation(th[:], cS[:, hc], AF.Tanh)
                nc.vector.tensor_tensor(hN[:, hc], sg(GATE_O)[:, hc], th[:], ALU.mult)
                for k in (2 * half, 2 * half + 1):
                    nc.tensor.matmul(
                        PT[:, 16 * k : 16 * k + 16],
                        hN[:, 128 * k : 128 * k + 128],
                        i16[:], start=True, stop=True,
                    )
                nc.vector.tensor_copy(
                    stage_out[:, 32 * half : 32 * half + 32],
                    PT[:, 32 * half : 32 * half + 32],
                )
                if seqbuf is not None:
                    nc.vector.tensor_copy(
                        seqbuf[:, 2 * half : 2 * half + 2, s16, :],
                        PT[:, 32 * half : 32 * half + 32].rearrange(
                            "p (k b) -> p k b", k=2
                        ),
                    )

        def emit_pass(src, wsb, usb, write_seq, seq_dst, eng):
            # preamble: dense chunk 0 -> slot 0 (static lhsD)
            lhsDs = lhp.tile([128, 4, 8, 16], F32R, name="lhsDs", tag="lhsDs")
            eng.dma_start(
                out=lhsDs[:],
                in_=src[0:4, :, 0:8, :].rearrange("k p a b -> p k a b"),
            )
            dense_emit(wsb, lhsDs, 0, 0, (0, 1, 2, 3))

            def body(i, last=False):
                # steps 0-7 read slot 0 (chunk 2i); dense chunk 2i+1 -> slot 1
                # steps 8-15 read slot 1 (chunk 2i+1); dense 2i+2 -> slot 0
                stat = isinstance(i, int)

                def tsl(start, n):
                    return slice(start, start + n) if stat else bass.ds(start, n)

                seqbuf = None
                if write_seq:
                    seqbuf = sqb.tile(
                        [128, 4, 16, 16], F32R, name="seqbuf", tag="sqb", bufs=1
                    )
                if last:
                    lhsD = lhp.tile([128, 4, 8, 16], F32R, name="lhsDs", tag="lhsDs")
                    eng.dma_start(
                        out=lhsD[:],
                        in_=src[0:4, :, 248:256, :].rearrange("k p a b -> p k a b"),
                    )
                else:
                    lhsD = lhp.tile(
                        [128, 4, 16, 16], F32R, name="lhsD", tag="lhsD", bufs=1
                    )
                    eng.dma_start(
                        out=lhsD[:],
                        in_=src[0:4, :, tsl(i * 16 + 8, 16), :].rearrange(
                            "k p a b -> p k a b"
                        ),
                    )
                for hb in range(2):
                    for s in range(8 * hb, 8 * hb + 8):
                        dj = None
                        if hb == 0:
                            arow, slot_d = 0, 1      # chunk 2i+1
                        else:
                            arow, slot_d = 8, 0      # chunk 2i+2
                        if hb == 0 or not last:
                            qsel = {1: (0,), 3: (1,), 5: (2,), 7: (3,)}.get(s % 8)
                            if qsel is not None:
                                dj = (wsb, lhsD, 0 if last else arow, slot_d, qsel)
                        step_emit(usb, s, seqbuf, dj)
                if write_seq:
                    eng.dma_start(
                        out=seq_dst[0:4, :, tsl(i * 16, 16), :].rearrange(
                            "k p u b -> p k u b"
                        ),
                        in_=seqbuf[:],
                    )

            with tc.For_i(0, 15, 1) as i:
                body(i)
            # last body (i=15): only first-half dense (chunk 31)
            body(15, last=True)

        # pass 0: src=xbT; passes 1..4: seqT ping-pong; pass 5: wsb2/usb2
        engs = [nc.sync, nc.scalar, nc.sync, nc.scalar, nc.sync, nc.scalar]
        emit_pass(xbT, w1sb, u1sb, True, seqT[0], engs[0])
        for p in range(1, 5):
            emit_pass(seqT[(p - 1) % 2], w1sb, u1sb, True, seqT[p % 2], engs[p])
        emit_pass(seqT[0], w2sb, u2sb, False, seqT[1], engs[5])

        nc.sync.dma_start(out=ht_out[:], in_=stages[par[0]][:])

        for pool in (dpool, tpool, ppool, sqb, lhp, hp, mp, sigp, wpool, dram):
            pool.release()

    split_multiwaits(nc)
    return nc


def make_inputs_for_core(x_core, W1, U1, b1, W2, U2, b2):
    Wb1, Ur1 = prep_weights(W1, U1, b1)
    Wb2, Ur2 = prep_weights(W2, U2, b2)
    return {
        "xbT": prep_xbT(x_core),
        "wb1": Wb1,
        "wb2": Wb2,
        "ur1": Ur1,
        "ur2": Ur2,
        "esel": make_esel(),
        "i16": np.eye(16, dtype=np.float32),
    }

'''

_mod = _types.ModuleType("kernel_embedded_lib")
exec(compile(_LIB_SOURCE, "kernel_embedded_lib", "exec"), _mod.__dict__)
sys.modules["kernel_embedded_lib"] = _mod

_BUILT = {}


def _build():
    if "nc" not in _BUILT:
        _BUILT["nc"] = _mod.build_kernel()
    return _BUILT["nc"]


def kernel(x, W1, U1, b1, W2, U2, b2):
    """Full inputs in, full output out. x [128,256,512] fp32 -> h [128,512]."""
    from concourse.bass_utils import run_bass_kernel_spmd

    x = np.asarray(x, dtype=np.float32)
    nc = _build()
    in_maps = []
    for c in range(8):
        in_maps.append(
            _mod.make_inputs_for_core(
                x[c * 16 : (c + 1) * 16],
                np.asarray(W1, np.float32),
                np.asarray(U1, np.float32),
                np.asarray(b1, np.float32),
                np.asarray(W2, np.float32),
                np.asarray(U2, np.float32),
                np.asarray(b2, np.float32),
            )
        )
    res = run_bass_kernel_spmd(nc, in_maps, list(range(8)))
    parts = [_mod.assemble_h(res.results[i]["ht_out"]) for i in range(8)]
    return np.concatenate(parts, axis=0)
